# revision 48
# baseline (speedup 1.0000x reference)
"""Trainium2 Bass kernel for DeiT self-attention with channel-pruning masks.

Reference computation (B=16, S=577, HID=768, H=12, D=64, N_KEEP=576):
    q/k/v = hs @ W + b            [B,S,576]
    scatter channels to [B,S,768] at {q,k,v}_idx, split into 12 heads of 64
    softmax attention per (b, h), concat heads, gather v_idx channels.

Strategy:
  - Host folds the q/k channel scatters into zero-padded weight matrices and
    packs V down to only the kept channels (plus one ones-column per head for
    the softmax denominator), so the device never computes discarded v
    channels. hs is pre-transposed per core on the host.
  - Data-parallel over batch: 8 cores x 2 images each (T = 1154 tokens/core).
  - All matmul inputs are bf16 (fp32 PSUM accumulation; rel err ~3e-3, and
    bf16 enables the PE's fast-weight-load path). Every matmul runs in the
    PE's (128,128) tile mode: k is stored per head with the off-head 64
    partition rows zeroed so score matmuls contract over the full 128 rows
    (tile-mode switches cost a ~100-250ns drain each, so uniformity wins
    over 64-row array packing).
  - Device schedule per core (single pass, software-pipelined so the ScalarE
    exp stream never starves):
      Q0/K0 projections first (small weight slices stream in early),
      V projection interleaved with chunk-0 img0 score blocks,
      then per chunk i (= heads 2i, 2i+1), per (head, image) unit:
        S^T = Kz_h^T x Q for both query tiles -> 2-bank PSUM tile
        E = exp(S^T / 8)   one ScalarE op per ktok chunk, PSUM -> SBUF
        ctxU^T|Z = [V_h|1]^T x E accumulated over ktok chunks
      with chunk i+1 projections and the previous chunk's ctx blocks
      interleaved between score blocks (ctx is carried one body downstream
      so the ACT stays busy across chunk boundaries).
  - Device output (bf16) rows per head: kept ctxU^T rows then Z (softmax
    denominators). Host divides, transposes, and reorders per-head blocks.
  - DMA dispatch queues serialize at ~700ns per descriptor, so inputs are
    spread across the sync/scalar/gpsimd queues and outputs across
    gpsimd/sync to keep head and tail latency down.
"""

import numpy as np

B, S, HID = 16, 577, 768
H, D = 12, 64
N_KEEP = 576
NCORES = 8
BPC = B // NCORES          # images per core
TOK = BPC * S              # tokens per core
VW = N_KEEP + H            # 588: kept V columns + one ones column per head
VW_PAD = 624               # v_sb column pad (65-wide per-head ctx lhsT slices)
P = 128
ICH = HID // P             # 6 input-channel chunks
OCH = HID // P             # 6 q/k output-channel chunks
TOK_TILES = [(0, 386), (386, 386), (772, 382)]      # projection moving tiles (even)
KCHUNKS = [(0, 128), (128, 128), (256, 128), (384, 128), (512, 65)]  # per image
# (q_offset, scores width (even), ctx width) per image; qt1 is shifted +289
QTILES = [(0, 290, 290), (289, 290, 288)]
TOK_P = 1160                                        # q/k token dim padded for qt1 reads

_NC_CACHE = {}


def _build_nc(use_f32r=True, msizes=None):
    import concourse.bacc as bacc
    import concourse.mybir as mybir
    import concourse.tile as tile

    f32 = mybir.dt.float32
    mm_dt = mybir.dt.bfloat16 if use_f32r else mybir.dt.float32

    # per-head augmented V block sizes (kept channels + 1 ones column) and
    # their column offsets in the packed [*, VW] V layout
    assert msizes is not None and sum(msizes) == VW
    assert all(32 < m <= 64 for m in msizes), msizes
    moffs = [sum(msizes[:h]) for h in range(H)]

    nc = bacc.Bacc("TRN2", target_bir_lowering=False)

    hsT = nc.dram_tensor("hsT", [HID, TOK], mm_dt, kind="ExternalInput")
    # host-swizzled: wq[p, i, c, n] = Wq_full[c*128+p, i*128+n] so each
    # chunk-i slice is one contiguous-descriptor DMA
    wq = nc.dram_tensor("wq", [P, OCH, ICH, P], mm_dt, kind="ExternalInput")
    wk = nc.dram_tensor("wk", [P, OCH, ICH, P], mm_dt, kind="ExternalInput")
    wv = nc.dram_tensor("wv", [HID, VW], mm_dt, kind="ExternalInput")
    bq = nc.dram_tensor("bq", [HID], f32, kind="ExternalInput")
    bk = nc.dram_tensor("bk", [HID], f32, kind="ExternalInput")
    bvb = nc.dram_tensor("bvb", [P, VW], f32, kind="ExternalInput")
    outA = nc.dram_tensor("outA", [VW, TOK], mm_dt, kind="ExternalOutput")

    def mm(out_ps, lhsT, rhs, start, stop):
        nc.tensor.matmul(out_ps, lhsT, rhs, start=start, stop=stop)

    with tile.TileContext(nc) as tc:
        Exp = mybir.ActivationFunctionType.Exp
        with (
            tc.tile_pool(name="big", bufs=1) as big,
            tc.tile_pool(name="psa", bufs=2, space="PSUM") as psa,   # proj accums + ctx chains
            tc.tile_pool(name="psb", bufs=3, space="PSUM") as psb,   # score pairs (2 banks each)
            tc.tile_pool(name="wpool", bufs=3) as wpool,
            tc.tile_pool(name="epool", bufs=8) as epool,
            tc.tile_pool(name="opool", bufs=4) as opool,
        ):
            # ---- persistent SBUF tensors ----
            # DMA order matters: the first V unit needs hsT piece (b0,j0) and
            # wv; later hsT pieces stream while V computes.
            hsT_sb = big.tile([P, ICH, TOK], mm_dt)
            hsT_r = hsT.rearrange("(c p) t -> p c t", p=P)
            bvb_sb = big.tile([P, VW], f32)
            bq_sb = big.tile([P, OCH], f32)
            bk_sb = big.tile([P, OCH], f32)

            q_sb = big.tile([P, OCH, TOK_P], mm_dt)
            # k stored per head with the other 64 partition rows zeroed: score
            # matmuls then use the full 128-row q chunk as rhs (the zero k
            # rows null the other head's contribution) so every matmul in the
            # kernel runs in the PE's (128, 128) mode - no tile-mode-switch
            # drains anywhere.
            kz_sb = big.tile([P, H, TOK], mm_dt)
            # v_sb column-padded so every head's 65-wide ctx lhsT slice stays
            # in bounds (the junk output rows are discarded before DMA)
            v_sb = big.tile([P, BPC * len(KCHUNKS), VW_PAD], mm_dt)
            # zero the padded token tail once (read by qt1 score matmuls for b=1)
            nc.vector.memset(q_sb[:, :, TOK:].bitcast(f32), 0.0)
            # zero v_sb's column pad tail (the 65-wide ctx lhsT slice of the
            # last head reads into it; junk must at least be finite)
            nc.vector.memset(v_sb[:, :, VW:].bitcast(f32), 0.0)
            # zero the off-head halves of kz_sb once (GpSimd: cheap, and the
            # DVE queue must stay clear for the first projection bias-adds)
            for h in range(H):
                pb = 64 * (h % 2)
                nc.gpsimd.memset(kz_sb[64 - pb : P - pb, h, :].bitcast(f32), 0.0)

            # ---- Q0/K0 projections first: they need only the small weight
            # slices + hsT, so the PE starts while the bulk inputs stream ----
            def load_w(i, w_dram):
                w_sb = wpool.tile([P, ICH, P], mm_dt, tag="w", name="w_sb")
                nc.sync.dma_start(w_sb[:], w_dram[:, i, :, :])
                return w_sb

            # head-latency-critical DMA order: wq0 split across two queues,
            # then the first-half hsT pieces round-robined over all three DMA
            # dispatch queues (~700ns serialization apiece), wk0 and the rest
            # behind them in need order.
            wq0 = wpool.tile([P, ICH, P], mm_dt, tag="w", name="w_sb")
            nc.sync.dma_start(wq0[:, 0:3, :], wq[:, 0, 0:3, :])
            nc.scalar.dma_start(wq0[:, 3:6, :], wq[:, 0, 3:6, :])
            wk0 = wpool.tile([P, ICH, P], mm_dt, tag="w", name="w_sb")
            HALF = S  # 577
            qs3 = (nc.sync, nc.scalar, nc.gpsimd)
            for c in range(ICH):
                qs3[c % 3].dma_start(hsT_sb[:, c, :HALF], hsT_r[:, c, :HALF])
            nc.scalar.dma_start(wk0[:], wk[:, 0, :, :])
            for c in range(ICH):
                qs3[c % 3].dma_start(
                    hsT_sb[:, c, HALF : 2 * HALF], hsT_r[:, c, HALF : 2 * HALF]
                )
            nc.sync.dma_start(bq_sb[:], bq.rearrange("(c p) -> p c", p=P))
            nc.sync.dma_start(bk_sb[:], bk.rearrange("(c p) -> p c", p=P))

            def emit_proj_t(i, w_sb, b_sb, dst, t):
                toff, tcs = TOK_TILES[t]
                qp = psa.tile([P, 512], f32, tag="ps", name="qp")[:, :tcs]
                for k in range(ICH):
                    mm(
                        qp,
                        w_sb[:, k, :],
                        hsT_sb[:, k, toff : toff + tcs],
                        start=(k == 0),
                        stop=(k == ICH - 1),
                    )
                if dst is None:
                    # k projection: scatter the two heads of chunk i into
                    # their per-head zero-padded kz_sb slots
                    for h, pb in ((2 * i, 0), (2 * i + 1, 64)):
                        nc.vector.tensor_add(
                            out=kz_sb[pb : pb + 64, h, toff : toff + tcs],
                            in0=qp[pb : pb + 64, :],
                            in1=b_sb[pb : pb + 64, i : i + 1].to_broadcast((64, tcs)),
                        )
                else:
                    nc.vector.tensor_add(
                        out=dst[:, i, toff : toff + tcs],
                        in0=qp,
                        in1=b_sb[:, i : i + 1].to_broadcast((P, tcs)),
                    )

            for t in range(3):
                emit_proj_t(0, wq0, bq_sb, q_sb, t)
            for t in range(3):
                emit_proj_t(0, wk0, bk_sb, None, t)

            NK = len(KCHUNKS)

            def emit_sp(i, h, b, e_sb, c):
                # scores for one (head, image) unit and one k-token chunk.
                # lhsT is the zero-padded per-head k slice, rhs the full
                # 128-row q chunk, so the matmul runs in (128, 128) mode.
                ko, kcs = KCHUNKS[c]
                sp2 = psb.tile([P, 1024], f32, tag="sp", name="sp2")
                for qt, (qo, sw, cw) in enumerate(QTILES):
                    mm(
                        sp2[:kcs, qt * 512 : qt * 512 + sw],
                        kz_sb[:, h, b * S + ko : b * S + ko + kcs],
                        q_sb[:, i, b * S + qo : b * S + qo + sw],
                        start=True,
                        stop=True,
                    )
                nc.scalar.activation(
                    e_sb[:kcs, c, :, :],
                    sp2.rearrange("p (two q) -> p two q", two=2)[:kcs, :, :290],
                    Exp,
                    scale=0.125,
                )

            ocur = {}  # (i, h, b) -> o_sb accumulating both query tiles

            def emit_cp2(i, h, b, e_sb, qt):
                # ctx accumulation chain over the k-token chunks. lhsT is the
                # head's packed V block padded to 65 columns (junk output rows
                # are discarded before the copy) so tile_size stays (128,128),
                # the same PE mode as the projections.
                m = msizes[h]
                off = moffs[h]
                qo, sw, cw = QTILES[qt]
                cp = psa.tile([P, 512], f32, tag="ps", name="cp")[:65, :cw]
                for c, (ko, kcs) in enumerate(KCHUNKS):
                    mm(cp, v_sb[:kcs, b * 5 + c, off : off + 65],
                       e_sb[:kcs, c, qt, :cw], start=(c == 0), stop=(c == NK - 1))
                # both query tiles land in one SBUF tile (qt0 at cols 0-288,
                # qt1 at 289-576) so each unit needs only ONE output DMA -
                # DMA dispatch queues serialize at ~700ns per descriptor
                if qt == 0:
                    o_sb = opool.tile([65, 580], mm_dt, tag="o", name="o_sb")
                    ocur[(i, h, b)] = o_sb
                    nc.vector.tensor_copy(o_sb[:m, :289], cp[:m, :289])
                else:
                    o_sb = ocur.pop((i, h, b))
                    nc.vector.tensor_copy(o_sb[:m, 289:577], cp[:m, :288])
                    ((nc.gpsimd if (h + b) % 2 == 0 else nc.sync)).dma_start(
                        outA[off : off + m, b * S : b * S + S],
                        o_sb[:m, :S],
                    )

            def alloc_es():
                return [
                    epool.tile([P, NK, 2, 290], mm_dt, tag="e", name="e_sb")
                    for _ in range(4)
                ]

            # ---- V projection, interleaved with chunk-0 img0 scores so the
            # ACT starts its exp stream early (wv lives only here) ----
            es0 = alloc_es()
            sp_hoist = [(u, c) for c in range(NK) for u in (0, 2)]
            with tc.tile_pool(name="pwv", bufs=1) as pwv:
                wv_sb = pwv.tile([P, ICH, VW], mm_dt)
                wv_r = wv.rearrange("(c p) n -> p c n", p=P)
                for k in range(ICH):
                    nc.gpsimd.dma_start(wv_sb[:, k, :], wv_r[:, k, :])
                nc.gpsimd.dma_start(bvb_sb[:], bvb[:])
                VT = VW // 2  # 294
                vunit = 0
                for b in range(BPC):
                    for j, (koff, kcs) in enumerate(KCHUNKS):
                        toff = b * S + koff
                        vps = [
                            psa.tile([P, 512], f32, tag="ps", name="vp")[:kcs, :VT]
                            for _ in range(2)
                        ]
                        for k in range(ICH):
                            for n in range(2):
                                mm(
                                    vps[n],
                                    hsT_sb[:, k, toff : toff + kcs],
                                    wv_sb[:, k, n * VT : (n + 1) * VT],
                                    start=(k == 0),
                                    stop=(k == ICH - 1),
                                )
                        for n in range(2):
                            nc.vector.tensor_add(
                                out=v_sb[:kcs, b * 5 + j, n * VT : (n + 1) * VT],
                                in0=vps[n],
                                in1=bvb_sb[:kcs, n * VT : (n + 1) * VT],
                            )
                        if vunit < len(sp_hoist):
                            u, c = sp_hoist[vunit]
                            emit_sp(0, u // 2, u % 2, es0[u], c)
                            vunit += 1

            # ---- interleaved Q/K projection + attention ----
            # Steady state per chunk body: this chunk's 20 score blocks spread
            # across the whole body (the ACT consumes one block every ~0.7us),
            # with the previous chunk's 8 ctx blocks and the next chunk's 6
            # projection tiles as PE fillers between them. All ctx blocks are
            # carried one body downstream so the ACT never starves at body
            # boundaries.
            prev = None  # (chunk, es) of previous chunk
            for i in range(OCH):
                es = es0 if i == 0 else alloc_es()
                last = i + 1 >= OCH
                wqn = None if last else load_w(i + 1, wq)
                wkn = None if last else load_w(i + 1, wk)

                def sp(u, c):
                    # units: 0 = (2i, img0), 1 = (2i, img1), 2 = (2i+1, img0),
                    # 3 = (2i+1, img1); chunk 0's img0 units ran in the V phase
                    if i == 0 and u in (0, 2):
                        return
                    emit_sp(i, 2 * i + u // 2, u % 2, es[u], c)

                def cpp(u, qt):
                    if prev is not None:
                        pi, pes = prev
                        emit_cp2(pi, 2 * pi + u // 2, u % 2, pes[u], qt)

                def pj(w_sb, b_sb, dst, t):
                    if not last:
                        emit_proj_t(i + 1, w_sb, b_sb, dst, t)

                def cpc(u, qt):
                    emit_cp2(i, 2 * i + u // 2, u % 2, es[u], qt)

                sp(0, 0)
                sp(2, 0)
                cpp(0, 0)
                sp(0, 1)
                sp(2, 1)
                cpp(0, 1)
                sp(0, 2)
                sp(2, 2)
                cpp(2, 0)
                sp(0, 3)
                sp(2, 3)
                cpp(2, 1)
                sp(0, 4)
                sp(2, 4)
                cpp(1, 0)
                pj(wqn, bq_sb, q_sb, 0)
                sp(1, 0)
                sp(3, 0)
                cpp(1, 1)
                pj(wqn, bq_sb, q_sb, 1)
                sp(1, 1)
                sp(3, 1)
                cpp(3, 0)
                pj(wqn, bq_sb, q_sb, 2)
                sp(1, 2)
                sp(3, 2)
                cpp(3, 1)
                pj(wkn, bk_sb, None, 0)
                sp(1, 3)
                sp(3, 3)
                pj(wkn, bk_sb, None, 1)
                if last:
                    # final chunk: its own ctx blocks drain inline as each
                    # unit's exps complete instead of all stalling at the end
                    cpc(0, 0)
                    sp(1, 4)
                    sp(3, 4)
                    cpc(0, 1)
                    cpc(2, 0)
                    cpc(2, 1)
                    cpc(1, 0)
                    cpc(1, 1)
                    cpc(3, 0)
                    cpc(3, 1)
                else:
                    sp(1, 4)
                    sp(3, 4)
                    pj(wkn, bk_sb, None, 2)
                prev = (i, es)

    nc.compile()
    return nc


def _get_nc(use_f32r=True, msizes=None):
    key = ("nc", use_f32r, msizes)
    if key not in _NC_CACHE:
        _NC_CACHE[key] = _build_nc(use_f32r, msizes)
    return _NC_CACHE[key]


def _make_in_maps(hidden_states, Wq, bq, Wk, bk, Wv, bv, q_idx, k_idx, v_idx,
                  use_f32r=True):
    f32 = np.float32
    hs = np.asarray(hidden_states, f32)
    q_idx = np.asarray(q_idx).astype(np.int64)
    k_idx = np.asarray(k_idx).astype(np.int64)
    v_idx = np.asarray(v_idx).astype(np.int64)

    # fold channel scatters into full-width weights
    wq_full = np.zeros((HID, HID), f32)
    wq_full[:, q_idx] = np.asarray(Wq, f32)
    bq_full = np.zeros(HID, f32)
    bq_full[q_idx] = np.asarray(bq, f32)
    wk_full = np.zeros((HID, HID), f32)
    wk_full[:, k_idx] = np.asarray(Wk, f32)
    bk_full = np.zeros(HID, f32)
    bk_full[k_idx] = np.asarray(bk, f32)

    # packed augmented V layout: per head the kept value columns (Wv columns
    # are already in sorted-v_idx order) + one ones column (softmax denom)
    Wv = np.asarray(Wv, f32)
    bv = np.asarray(bv, f32)
    kept = np.bincount(v_idx // D, minlength=H)
    msizes = tuple(int(k) + 1 for k in kept)
    wv_aug = np.zeros((HID, VW), f32)
    bv_aug = np.zeros(VW, f32)
    cum = 0
    moff = 0
    for h in range(H):
        kh = int(kept[h])
        wv_aug[:, moff : moff + kh] = Wv[:, cum : cum + kh]
        bv_aug[moff : moff + kh] = bv[cum : cum + kh]
        bv_aug[moff + kh] = 1.0
        cum += kh
        moff += kh + 1
    bvb = np.broadcast_to(bv_aug, (P, VW)).copy()

    if use_f32r:
        import ml_dtypes

        bf16 = ml_dtypes.bfloat16
        wq_full = wq_full.astype(bf16)
        wk_full = wk_full.astype(bf16)
        wv_aug = wv_aug.astype(bf16)
    # swizzle projection weights to [p, i, c, n] (slice-contiguous DMA layout)
    wq_full = np.ascontiguousarray(
        wq_full.reshape(ICH, P, OCH, P).transpose(1, 2, 0, 3)
    )
    wk_full = np.ascontiguousarray(
        wk_full.reshape(ICH, P, OCH, P).transpose(1, 2, 0, 3)
    )

    in_maps = []
    for c in range(NCORES):
        hsT = np.ascontiguousarray(
            hs[c * BPC : (c + 1) * BPC].reshape(TOK, HID).T
        )
        if use_f32r:
            hsT = hsT.astype(bf16)
        in_maps.append(
            {
                "hsT": hsT,
                "wq": wq_full,
                "wk": wk_full,
                "wv": wv_aug,
                "bq": bq_full,
                "bk": bk_full,
                "bvb": bvb,
            }
        )
    return in_maps, msizes


def _assemble_output(results, msizes):
    ctx = np.empty((B, S, N_KEEP), np.float32)
    vals = np.empty((N_KEEP, TOK), np.float32)
    for c in range(NCORES):
        aug = np.asarray(results[c]["outA"], np.float32)  # [VW, TOK]
        cum = 0
        moff = 0
        for h in range(H):
            kh = msizes[h] - 1
            vals[cum : cum + kh] = aug[moff : moff + kh] / aug[moff + kh]
            cum += kh
            moff += kh + 1
        ctx[c * BPC : (c + 1) * BPC] = vals.T.reshape(BPC, S, N_KEEP)
    return np.ascontiguousarray(ctx)


def run(inputs, trace=False, use_f32r=True, **spmd_kwargs):
    """Full pipeline; returns (output, BassKernelResults)."""
    from concourse import bass_utils

    in_maps, msizes = _make_in_maps(**inputs, use_f32r=use_f32r)
    nc = _get_nc(use_f32r, msizes)
    res = bass_utils.run_bass_kernel_spmd(
        nc, in_maps, core_ids=list(range(NCORES)), trace=trace, **spmd_kwargs
    )
    return _assemble_output(res.results, msizes), res


def kernel(**inputs):
    out, _ = run(inputs, trace=False)
    return out



# revision 49
# speedup vs baseline: 1.0173x; 1.0173x over previous
"""Trainium2 Bass kernel for DeiT self-attention with channel-pruning masks.

Reference computation (B=16, S=577, HID=768, H=12, D=64, N_KEEP=576):
    q/k/v = hs @ W + b            [B,S,576]
    scatter channels to [B,S,768] at {q,k,v}_idx, split into 12 heads of 64
    softmax attention per (b, h), concat heads, gather v_idx channels.

Strategy:
  - Host folds the q/k channel scatters into zero-padded weight matrices and
    packs V down to only the kept channels (plus one ones-column per head for
    the softmax denominator), so the device never computes discarded v
    channels. hs is pre-transposed per core on the host.
  - Data-parallel over batch: 8 cores x 2 images each (T = 1154 tokens/core).
  - All matmul inputs are bf16 (fp32 PSUM accumulation; rel err ~3e-3, and
    bf16 enables the PE's fast-weight-load path). Every matmul runs in the
    PE's (128,128) tile mode: k is stored per head with the off-head 64
    partition rows zeroed so score matmuls contract over the full 128 rows
    (tile-mode switches cost a ~100-250ns drain each, so uniformity wins
    over 64-row array packing).
  - Device schedule per core (single pass, software-pipelined so the ScalarE
    exp stream never starves):
      Q0/K0 projections first (small weight slices stream in early),
      V projection interleaved with chunk-0 img0 score blocks,
      then per chunk i (= heads 2i, 2i+1), per (head, image) unit:
        S^T = Kz_h^T x Q for both query tiles -> 2-bank PSUM tile
        E = exp(S^T / 8)   one ScalarE op per ktok chunk, PSUM -> SBUF
        ctxU^T|Z = [V_h|1]^T x E accumulated over ktok chunks
      with chunk i+1 projections and the previous chunk's ctx blocks
      interleaved between score blocks (ctx is carried one body downstream
      so the ACT stays busy across chunk boundaries).
  - Device output (bf16) rows per head: kept ctxU^T rows then Z (softmax
    denominators). Host divides, transposes, and reorders per-head blocks.
  - DMA dispatch queues serialize at ~700ns per descriptor, so inputs are
    spread across the sync/scalar/gpsimd queues and outputs across
    gpsimd/sync to keep head and tail latency down.
"""

import numpy as np

B, S, HID = 16, 577, 768
H, D = 12, 64
N_KEEP = 576
NCORES = 8
BPC = B // NCORES          # images per core
TOK = BPC * S              # tokens per core
VW = N_KEEP + H            # 588: kept V columns + one ones column per head
VW_PAD = 624               # v_sb column pad (65-wide per-head ctx lhsT slices)
P = 128
ICH = HID // P             # 6 input-channel chunks
OCH = HID // P             # 6 q/k output-channel chunks
TOK_TILES = [(0, 386), (386, 386), (772, 382)]      # projection moving tiles (even)
KCHUNKS = [(0, 128), (128, 128), (256, 128), (384, 128), (512, 65)]  # per image
# (q_offset, scores width (even), ctx width) per image; qt1 is shifted +289
QTILES = [(0, 290, 290), (289, 290, 288)]
TOK_P = 1160                                        # q/k token dim padded for qt1 reads

_NC_CACHE = {}


def _build_nc(use_f32r=True, msizes=None):
    import concourse.bacc as bacc
    import concourse.mybir as mybir
    import concourse.tile as tile

    f32 = mybir.dt.float32
    mm_dt = mybir.dt.bfloat16 if use_f32r else mybir.dt.float32

    # per-head augmented V block sizes (kept channels + 1 ones column) and
    # their column offsets in the packed [*, VW] V layout
    assert msizes is not None and sum(msizes) == VW
    assert all(32 < m <= 64 for m in msizes), msizes
    moffs = [sum(msizes[:h]) for h in range(H)]

    nc = bacc.Bacc("TRN2", target_bir_lowering=False)

    hsT = nc.dram_tensor("hsT", [HID, TOK], mm_dt, kind="ExternalInput")
    # host-swizzled: wq[p, i, c, n] = Wq_full[c*128+p, i*128+n] so each
    # chunk-i slice is one contiguous-descriptor DMA
    wq = nc.dram_tensor("wq", [P, OCH, ICH, P], mm_dt, kind="ExternalInput")
    wk = nc.dram_tensor("wk", [P, OCH, ICH, P], mm_dt, kind="ExternalInput")
    wv = nc.dram_tensor("wv", [HID, VW], mm_dt, kind="ExternalInput")
    bq = nc.dram_tensor("bq", [HID], f32, kind="ExternalInput")
    bk = nc.dram_tensor("bk", [HID], f32, kind="ExternalInput")
    bvb = nc.dram_tensor("bvb", [P, VW], f32, kind="ExternalInput")
    outA = nc.dram_tensor("outA", [VW, TOK], mm_dt, kind="ExternalOutput")

    def mm(out_ps, lhsT, rhs, start, stop):
        nc.tensor.matmul(out_ps, lhsT, rhs, start=start, stop=stop)

    with tile.TileContext(nc) as tc:
        Exp = mybir.ActivationFunctionType.Exp
        with (
            tc.tile_pool(name="big", bufs=1) as big,
            tc.tile_pool(name="psa", bufs=2, space="PSUM") as psa,   # proj accums + ctx chains
            tc.tile_pool(name="psb", bufs=3, space="PSUM") as psb,   # score pairs (2 banks each)
            tc.tile_pool(name="wpool", bufs=3) as wpool,
            tc.tile_pool(name="epool", bufs=8) as epool,
            tc.tile_pool(name="opool", bufs=4) as opool,
        ):
            # ---- persistent SBUF tensors ----
            # DMA order matters: the first V unit needs hsT piece (b0,j0) and
            # wv; later hsT pieces stream while V computes.
            hsT_sb = big.tile([P, ICH, TOK], mm_dt)
            hsT_r = hsT.rearrange("(c p) t -> p c t", p=P)
            bvb_sb = big.tile([P, VW], f32)
            bq_sb = big.tile([P, OCH], f32)
            bk_sb = big.tile([P, OCH], f32)

            q_sb = big.tile([P, OCH, TOK_P], mm_dt)
            # k stored per head with the other 64 partition rows zeroed: score
            # matmuls then use the full 128-row q chunk as rhs (the zero k
            # rows null the other head's contribution) so every matmul in the
            # kernel runs in the PE's (128, 128) mode - no tile-mode-switch
            # drains anywhere.
            kz_sb = big.tile([P, H, TOK], mm_dt)
            # v_sb column-padded so every head's 65-wide ctx lhsT slice stays
            # in bounds (the junk output rows are discarded before DMA)
            v_sb = big.tile([P, BPC * len(KCHUNKS), VW_PAD], mm_dt)
            # zero the padded token tail once (read by qt1 score matmuls for b=1)
            nc.vector.memset(q_sb[:, :, TOK:].bitcast(f32), 0.0)
            # zero v_sb's column pad tail (the 65-wide ctx lhsT slice of the
            # last head reads into it; junk must at least be finite)
            nc.vector.memset(v_sb[:, :, VW:].bitcast(f32), 0.0)
            # zero the off-head halves of kz_sb once (GpSimd: cheap, and the
            # DVE queue must stay clear for the first projection bias-adds)
            for h in range(H):
                pb = 64 * (h % 2)
                nc.gpsimd.memset(kz_sb[64 - pb : P - pb, h, :].bitcast(f32), 0.0)

            # ---- Q0/K0 projections first: they need only the small weight
            # slices + hsT, so the PE starts while the bulk inputs stream ----
            def load_w(i, w_dram):
                w_sb = wpool.tile([P, ICH, P], mm_dt, tag="w", name="w_sb")
                nc.sync.dma_start(w_sb[:], w_dram[:, i, :, :])
                return w_sb

            # head-latency-critical DMA order: wq0 split across two queues,
            # then the first-half hsT pieces round-robined over all three DMA
            # dispatch queues (~700ns serialization apiece), wk0 and the rest
            # behind them in need order.
            wq0 = wpool.tile([P, ICH, P], mm_dt, tag="w", name="w_sb")
            nc.sync.dma_start(wq0[:, 0:3, :], wq[:, 0, 0:3, :])
            nc.scalar.dma_start(wq0[:, 3:6, :], wq[:, 0, 3:6, :])
            wk0 = wpool.tile([P, ICH, P], mm_dt, tag="w", name="w_sb")
            HALF = S  # 577
            qs3 = (nc.sync, nc.scalar, nc.gpsimd)
            for c in range(ICH):
                qs3[c % 3].dma_start(hsT_sb[:, c, :HALF], hsT_r[:, c, :HALF])
            nc.scalar.dma_start(wk0[:], wk[:, 0, :, :])
            for c in range(ICH):
                qs3[c % 3].dma_start(
                    hsT_sb[:, c, HALF : 2 * HALF], hsT_r[:, c, HALF : 2 * HALF]
                )
            nc.sync.dma_start(bq_sb[:], bq.rearrange("(c p) -> p c", p=P))
            nc.sync.dma_start(bk_sb[:], bk.rearrange("(c p) -> p c", p=P))

            def emit_proj_t(i, w_sb, b_sb, dst, t):
                toff, tcs = TOK_TILES[t]
                qp = psa.tile([P, 512], f32, tag="ps", name="qp")[:, :tcs]
                for k in range(ICH):
                    mm(
                        qp,
                        w_sb[:, k, :],
                        hsT_sb[:, k, toff : toff + tcs],
                        start=(k == 0),
                        stop=(k == ICH - 1),
                    )
                if dst is None:
                    # k projection: scatter the two heads of chunk i into
                    # their per-head zero-padded kz_sb slots
                    for h, pb in ((2 * i, 0), (2 * i + 1, 64)):
                        nc.vector.tensor_add(
                            out=kz_sb[pb : pb + 64, h, toff : toff + tcs],
                            in0=qp[pb : pb + 64, :],
                            in1=b_sb[pb : pb + 64, i : i + 1].to_broadcast((64, tcs)),
                        )
                else:
                    nc.vector.tensor_add(
                        out=dst[:, i, toff : toff + tcs],
                        in0=qp,
                        in1=b_sb[:, i : i + 1].to_broadcast((P, tcs)),
                    )

            for t in range(3):
                emit_proj_t(0, wq0, bq_sb, q_sb, t)
            for t in range(3):
                emit_proj_t(0, wk0, bk_sb, None, t)

            NK = len(KCHUNKS)

            def emit_sp(i, h, b, e_sb, c):
                # scores for one (head, image) unit and one k-token chunk.
                # lhsT is the zero-padded per-head k slice, rhs the full
                # 128-row q chunk, so the matmul runs in (128, 128) mode.
                ko, kcs = KCHUNKS[c]
                sp2 = psb.tile([P, 1024], f32, tag="sp", name="sp2")
                for qt, (qo, sw, cw) in enumerate(QTILES):
                    mm(
                        sp2[:kcs, qt * 512 : qt * 512 + sw],
                        kz_sb[:, h, b * S + ko : b * S + ko + kcs],
                        q_sb[:, i, b * S + qo : b * S + qo + sw],
                        start=True,
                        stop=True,
                    )
                nc.scalar.activation(
                    e_sb[:kcs, c, :, :],
                    sp2.rearrange("p (two q) -> p two q", two=2)[:kcs, :, :290],
                    Exp,
                    scale=0.125,
                )

            def emit_cp2(i, h, b, e_sb, qt):
                # ctx accumulation chain over the k-token chunks. lhsT is the
                # head's packed V block padded to 65 columns (junk output rows
                # are discarded before the copy) so tile_size stays (128,128),
                # the same PE mode as the projections.
                m = msizes[h]
                off = moffs[h]
                qo, sw, cw = QTILES[qt]
                cp = psa.tile([P, 512], f32, tag="ps", name="cp")[:65, :cw]
                for c, (ko, kcs) in enumerate(KCHUNKS):
                    mm(cp, v_sb[:kcs, b * 5 + c, off : off + 65],
                       e_sb[:kcs, c, qt, :cw], start=(c == 0), stop=(c == NK - 1))
                o_sb = opool.tile([65, 512], mm_dt, tag="o", name="o_sb")
                nc.vector.tensor_copy(o_sb[:m, :cw], cp[:m, :])
                ow = cw if qt == 1 else 289
                # output DMAs split across the GpSimd and sync queues (each
                # queue serializes at ~700ns per descriptor; splitting halves
                # the end-of-kernel drain backlog)
                (nc.gpsimd if qt == 0 else nc.sync).dma_start(
                    outA[off : off + m, b * S + qo : b * S + qo + ow],
                    o_sb[:m, :ow],
                )

            def alloc_es():
                return [
                    epool.tile([P, NK, 2, 290], mm_dt, tag="e", name="e_sb")
                    for _ in range(4)
                ]

            # ---- V projection, interleaved with chunk-0 img0 scores so the
            # ACT starts its exp stream early (wv lives only here) ----
            es0 = alloc_es()
            sp_hoist = [(u, c) for c in range(NK) for u in (0, 2)]
            with tc.tile_pool(name="pwv", bufs=1) as pwv:
                wv_sb = pwv.tile([P, ICH, VW], mm_dt)
                wv_r = wv.rearrange("(c p) n -> p c n", p=P)
                for k in range(ICH):
                    nc.gpsimd.dma_start(wv_sb[:, k, :], wv_r[:, k, :])
                nc.gpsimd.dma_start(bvb_sb[:], bvb[:])
                VT = VW // 2  # 294
                vunit = 0
                for b in range(BPC):
                    for j, (koff, kcs) in enumerate(KCHUNKS):
                        toff = b * S + koff
                        vps = [
                            psa.tile([P, 512], f32, tag="ps", name="vp")[:kcs, :VT]
                            for _ in range(2)
                        ]
                        for k in range(ICH):
                            for n in range(2):
                                mm(
                                    vps[n],
                                    hsT_sb[:, k, toff : toff + kcs],
                                    wv_sb[:, k, n * VT : (n + 1) * VT],
                                    start=(k == 0),
                                    stop=(k == ICH - 1),
                                )
                        for n in range(2):
                            nc.vector.tensor_add(
                                out=v_sb[:kcs, b * 5 + j, n * VT : (n + 1) * VT],
                                in0=vps[n],
                                in1=bvb_sb[:kcs, n * VT : (n + 1) * VT],
                            )
                        if vunit < len(sp_hoist):
                            u, c = sp_hoist[vunit]
                            emit_sp(0, u // 2, u % 2, es0[u], c)
                            vunit += 1

            # ---- interleaved Q/K projection + attention ----
            # Steady state per chunk body: this chunk's 20 score blocks spread
            # across the whole body (the ACT consumes one block every ~0.7us),
            # with the previous chunk's 8 ctx blocks and the next chunk's 6
            # projection tiles as PE fillers between them. All ctx blocks are
            # carried one body downstream so the ACT never starves at body
            # boundaries.
            prev = None  # (chunk, es) of previous chunk
            for i in range(OCH):
                es = es0 if i == 0 else alloc_es()
                last = i + 1 >= OCH
                wqn = None if last else load_w(i + 1, wq)
                wkn = None if last else load_w(i + 1, wk)

                def sp(u, c):
                    # units: 0 = (2i, img0), 1 = (2i, img1), 2 = (2i+1, img0),
                    # 3 = (2i+1, img1); chunk 0's img0 units ran in the V phase
                    if i == 0 and u in (0, 2):
                        return
                    emit_sp(i, 2 * i + u // 2, u % 2, es[u], c)

                def cpp(u, qt):
                    if prev is not None:
                        pi, pes = prev
                        emit_cp2(pi, 2 * pi + u // 2, u % 2, pes[u], qt)

                def pj(w_sb, b_sb, dst, t):
                    if not last:
                        emit_proj_t(i + 1, w_sb, b_sb, dst, t)

                def cpc(u, qt):
                    emit_cp2(i, 2 * i + u // 2, u % 2, es[u], qt)

                sp(0, 0)
                sp(2, 0)
                cpp(0, 0)
                sp(0, 1)
                sp(2, 1)
                cpp(0, 1)
                sp(0, 2)
                sp(2, 2)
                cpp(2, 0)
                sp(0, 3)
                sp(2, 3)
                cpp(2, 1)
                sp(0, 4)
                sp(2, 4)
                cpp(1, 0)
                pj(wqn, bq_sb, q_sb, 0)
                sp(1, 0)
                sp(3, 0)
                cpp(1, 1)
                pj(wqn, bq_sb, q_sb, 1)
                sp(1, 1)
                sp(3, 1)
                cpp(3, 0)
                pj(wqn, bq_sb, q_sb, 2)
                sp(1, 2)
                sp(3, 2)
                cpp(3, 1)
                pj(wkn, bk_sb, None, 0)
                sp(1, 3)
                sp(3, 3)
                pj(wkn, bk_sb, None, 1)
                if last:
                    # final chunk: its own ctx blocks drain inline as each
                    # unit's exps complete instead of all stalling at the end
                    cpc(0, 0)
                    sp(1, 4)
                    sp(3, 4)
                    cpc(0, 1)
                    cpc(2, 0)
                    cpc(2, 1)
                    cpc(1, 0)
                    cpc(1, 1)
                    cpc(3, 0)
                    cpc(3, 1)
                else:
                    sp(1, 4)
                    sp(3, 4)
                    pj(wkn, bk_sb, None, 2)
                prev = (i, es)

    nc.compile()
    return nc


def _get_nc(use_f32r=True, msizes=None):
    key = ("nc", use_f32r, msizes)
    if key not in _NC_CACHE:
        _NC_CACHE[key] = _build_nc(use_f32r, msizes)
    return _NC_CACHE[key]


def _make_in_maps(hidden_states, Wq, bq, Wk, bk, Wv, bv, q_idx, k_idx, v_idx,
                  use_f32r=True):
    f32 = np.float32
    hs = np.asarray(hidden_states, f32)
    q_idx = np.asarray(q_idx).astype(np.int64)
    k_idx = np.asarray(k_idx).astype(np.int64)
    v_idx = np.asarray(v_idx).astype(np.int64)

    # fold channel scatters into full-width weights
    wq_full = np.zeros((HID, HID), f32)
    wq_full[:, q_idx] = np.asarray(Wq, f32)
    bq_full = np.zeros(HID, f32)
    bq_full[q_idx] = np.asarray(bq, f32)
    wk_full = np.zeros((HID, HID), f32)
    wk_full[:, k_idx] = np.asarray(Wk, f32)
    bk_full = np.zeros(HID, f32)
    bk_full[k_idx] = np.asarray(bk, f32)

    # packed augmented V layout: per head the kept value columns (Wv columns
    # are already in sorted-v_idx order) + one ones column (softmax denom)
    Wv = np.asarray(Wv, f32)
    bv = np.asarray(bv, f32)
    kept = np.bincount(v_idx // D, minlength=H)
    msizes = tuple(int(k) + 1 for k in kept)
    wv_aug = np.zeros((HID, VW), f32)
    bv_aug = np.zeros(VW, f32)
    cum = 0
    moff = 0
    for h in range(H):
        kh = int(kept[h])
        wv_aug[:, moff : moff + kh] = Wv[:, cum : cum + kh]
        bv_aug[moff : moff + kh] = bv[cum : cum + kh]
        bv_aug[moff + kh] = 1.0
        cum += kh
        moff += kh + 1
    bvb = np.broadcast_to(bv_aug, (P, VW)).copy()

    if use_f32r:
        import ml_dtypes

        bf16 = ml_dtypes.bfloat16
        wq_full = wq_full.astype(bf16)
        wk_full = wk_full.astype(bf16)
        wv_aug = wv_aug.astype(bf16)
    # swizzle projection weights to [p, i, c, n] (slice-contiguous DMA layout)
    wq_full = np.ascontiguousarray(
        wq_full.reshape(ICH, P, OCH, P).transpose(1, 2, 0, 3)
    )
    wk_full = np.ascontiguousarray(
        wk_full.reshape(ICH, P, OCH, P).transpose(1, 2, 0, 3)
    )

    in_maps = []
    for c in range(NCORES):
        hsT = np.ascontiguousarray(
            hs[c * BPC : (c + 1) * BPC].reshape(TOK, HID).T
        )
        if use_f32r:
            hsT = hsT.astype(bf16)
        in_maps.append(
            {
                "hsT": hsT,
                "wq": wq_full,
                "wk": wk_full,
                "wv": wv_aug,
                "bq": bq_full,
                "bk": bk_full,
                "bvb": bvb,
            }
        )
    return in_maps, msizes


def _assemble_output(results, msizes):
    ctx = np.empty((B, S, N_KEEP), np.float32)
    vals = np.empty((N_KEEP, TOK), np.float32)
    for c in range(NCORES):
        aug = np.asarray(results[c]["outA"], np.float32)  # [VW, TOK]
        cum = 0
        moff = 0
        for h in range(H):
            kh = msizes[h] - 1
            vals[cum : cum + kh] = aug[moff : moff + kh] / aug[moff + kh]
            cum += kh
            moff += kh + 1
        ctx[c * BPC : (c + 1) * BPC] = vals.T.reshape(BPC, S, N_KEEP)
    return np.ascontiguousarray(ctx)


def run(inputs, trace=False, use_f32r=True, **spmd_kwargs):
    """Full pipeline; returns (output, BassKernelResults)."""
    from concourse import bass_utils

    in_maps, msizes = _make_in_maps(**inputs, use_f32r=use_f32r)
    nc = _get_nc(use_f32r, msizes)
    res = bass_utils.run_bass_kernel_spmd(
        nc, in_maps, core_ids=list(range(NCORES)), trace=trace, **spmd_kwargs
    )
    return _assemble_output(res.results, msizes), res


def kernel(**inputs):
    out, _ = run(inputs, trace=False)
    return out

